# revision 17
# baseline (speedup 1.0000x reference)
"""Trainium2 Bass kernel for BeatDetectionRSNN2 (2-layer integrate-and-fire RSNN).

Reference semantics (per time step t):
    v1 += x_t @ W1.T ; s1 = (v1 >= 1); v1 *= (1 - s1)
    v2 += s1 @ W2.T  ; s2 = (v2 >= 1); v2 *= (1 - s2)
    out[:, t, :] = s2

Sharding: data-parallel over batch across 8 cores (16 batch rows each),
weights replicated, time recurrence local per core.

Per-core plan (all shapes hardcoded: B_c=16, T=4096, D=128, O=2):
  - PE computes u1 = x @ W1.T ahead of time in K-step chunks
    (x tiles [t,d] are PE-transposed to [d,t]; one big matmul per chunk).
  - The serial recurrence runs on the vector engine as 2 fused ops per
    step over a [128, 17] tile: cols 0..15 = v1 laid out [d, b], col 16
    = v2 for the 32 (b,o) pairs on partitions 0..31 (layer-2 fused into
    the same instructions, consuming u2 produced 2 chunks behind).
        I1: w = v + u          (tensor_tensor add)
        I2: v = (w < 1) * w    (scalar_tensor_tensor)
  - Spikes s1 = (w >= 1) are extracted per chunk (gpsimd) and fed to PE
    for u2 = s1 @ W2.T; s2 comes from col 16 of w.
"""
import sys
import numpy as np

if '/opt/trn_rl_repo' not in sys.path:
    sys.path.insert(0, '/opt/trn_rl_repo')

import concourse.bacc as bacc
import concourse.tile as tile
import concourse.mybir as mybir
from concourse.masks import make_identity
from concourse.bass_utils import run_bass_kernel_spmd

f32 = mybir.dt.float32
Alu = mybir.AluOpType

B, T, D, O = 128, 4096, 128, 2
NCORES = 8
BC = B // NCORES          # 16 batch rows per core
K = 128                   # chunk (time steps)
NC = T // K               # 32 chunks
FD = BC + 1               # 17 chain columns


def build_program(t_steps=T):
    nch = t_steps // K
    nc = bacc.Bacc("TRN2", target_bir_lowering=False)
    x_ext = nc.declare_dram_parameter("x", [BC, t_steps, D], f32, isOutput=False)
    w1t_ext = nc.declare_dram_parameter("w1t", [D, D], f32, isOutput=False)
    w2t_ext = nc.declare_dram_parameter("w2t", [D, O], f32, isOutput=False)
    # output stored (o, b, t) so the per-chunk DMA from [32, K] staging
    # (partition p = 16*o + b) is contiguous; host transposes to [b, t, o].
    out_ext = nc.declare_dram_parameter("out", [O, BC, t_steps], f32, isOutput=True)

    with tile.TileContext(nc) as tc:
        with (
            tc.tile_pool(name="consts", bufs=1) as consts,
            tc.tile_pool(name="xin", bufs=6) as xin_pool,
            tc.tile_pool(name="xT", bufs=2) as xT_pool,
            tc.tile_pool(name="ubuf", bufs=4) as u_pool,
            tc.tile_pool(name="wbuf", bufs=3) as w_pool,
            tc.tile_pool(name="gbuf", bufs=2) as g_pool,
            tc.tile_pool(name="s2st", bufs=3) as s2_pool,
            tc.tile_pool(name="u2ep", bufs=2) as u2ep_pool,
            tc.tile_pool(name="u2sb", bufs=2) as u2sb_pool,
            tc.tile_pool(name="u2c", bufs=3) as u2c_pool,
            tc.tile_pool(name="wep", bufs=2) as wep_pool,
            tc.tile_pool(name="xpose", bufs=2, space="PSUM") as xpose_pool,
            tc.tile_pool(name="upsum", bufs=2, space="PSUM") as upsum_pool,
            tc.tile_pool(name="u2psum", bufs=1, space="PSUM") as u2psum_pool,
        ):
            ident = consts.tile([128, 128], f32)
            make_identity(nc, ident[:])
            w1t = consts.tile([D, D], f32)
            w2t = consts.tile([D, O], f32)
            v_all = consts.tile([128, FD], f32)
            nc.sync.dma_start(w1t[:], w1t_ext[:])
            nc.sync.dma_start(w2t[:], w2t_ext[:])
            nc.vector.memset(v_all[:], 0.0)

            # pre-create U chunk tiles (u2 writes target chunk c+2)
            u_tiles = [u_pool.tile([128, FD * K], f32, tag="ubuf", name=f"u_c{c}")
                       for c in range(nch)]
            u2ep_tiles = [u2ep_pool.tile([32, K], f32, tag="u2ep", name=f"u2ep{e}")
                          for e in range(2)]

            for c in range(nch):
                u_t = u_tiles[c]
                if c < 2:
                    # layer-2 inputs for the first two chunks are zero
                    nc.vector.memset(u_t[0:32, BC::FD], 0.0)

                # ---- produce u1 for chunk c ----
                xT = xT_pool.tile([128, BC * K], f32, tag="xT")
                for j in range(4):
                    xp = xpose_pool.tile([128, 4, 128], f32, tag="xpose")
                    for i in range(4):
                        b = 4 * j + i
                        xt = xin_pool.tile([128, 128], f32, tag="xin")
                        nc.sync.dma_start(xt[:], x_ext[b, c * K:(c + 1) * K, :])
                        nc.tensor.transpose(xp[:, i, :], xt[:], ident[:])
                    nc.scalar.copy(xT[:, j * 512:(j + 1) * 512], xp[:])
                for j in range(4):
                    up = upsum_pool.tile([128, 512], f32, tag="upsum")
                    nc.tensor.matmul(up[:], w1t[:], xT[:, j * 512:(j + 1) * 512],
                                     start=True, stop=True)
                    # copy u1 psum -> U chunk cols {t*FD + b}, b in [4j, 4j+4)
                    dst = u_t[:].rearrange("p (t f) -> p f t", f=FD)[:, 4 * j:4 * j + 4, :]
                    src = up[:].rearrange("p (b t) -> p b t", b=4)
                    nc.scalar.copy(dst, src)

                # ---- serial chain for chunk c ----
                w_t = w_pool.tile([128, FD * K], f32, tag="wbuf")
                for t in range(K):
                    sl = slice(t * FD, t * FD + FD)
                    nc.vector.tensor_tensor(out=w_t[:, sl], in0=v_all[:],
                                            in1=u_t[:, sl], op=Alu.add)
                    nc.vector.scalar_tensor_tensor(out=v_all[:], in0=w_t[:, sl],
                                                   scalar=1.0, in1=w_t[:, sl],
                                                   op0=Alu.is_lt, op1=Alu.mult)

                # ---- spikes s1 -> g (b-major [128, b*K+t]) ----
                g_t = g_pool.tile([128, BC * K], f32, tag="gbuf")
                g3 = g_t[:].rearrange("p (b t) -> p b t", b=BC)
                w3 = w_t[:].rearrange("p (t f) -> p f t", f=FD)[:, 0:BC, :]
                nc.gpsimd.tensor_scalar(g3, w3, 1.0, None, op0=Alu.is_ge)

                # ---- layer-2 matmuls: u2 psum [2(o), b*K+t]; then DMA remap to
                # col-16 partitions p = 16*o + b (linear element orders match).
                u2p = u2psum_pool.tile([2, BC * K], f32, tag="u2psum")
                for j in range(4):
                    nc.tensor.matmul(u2p[:, j * 512:(j + 1) * 512], w2t[:],
                                     g_t[:, j * 512:(j + 1) * 512], start=True, stop=True)
                u2s = u2sb_pool.tile([2, BC * K], f32, tag="u2sb")
                nc.scalar.copy(u2s[:], u2p[:])
                u2c = u2c_pool.tile([32, K], f32, tag="u2c")
                nc.sync.dma_start(u2c[:], u2s[:])
                if c + 2 < nch:
                    nc.scalar.copy(u_tiles[c + 2][0:32, BC::FD], u2c[:])
                else:
                    nc.scalar.copy(u2ep_tiles[c + 2 - nch][:], u2c[:])

                # ---- s2 output for time-chunk tau = c - 2 ----
                if c >= 2:
                    tau = c - 2
                    s2 = s2_pool.tile([32, K], f32, tag="s2st")
                    nc.vector.tensor_scalar(s2[:], w_t[0:32, BC::FD], 1.0, None,
                                            op0=Alu.is_ge)
                    nc.sync.dma_start(out_ext[:, :, tau * K:(tau + 1) * K], s2[:])

            # ---- epilogue: layer-2 for the last two chunks ----
            for e in range(2):
                tau = nch - 2 + e
                wep = wep_pool.tile([32, K], f32, tag="wep")
                for t in range(K):
                    nc.vector.tensor_tensor(out=wep[:, t:t + 1], in0=v_all[0:32, BC:BC + 1],
                                            in1=u2ep_tiles[e][:, t:t + 1], op=Alu.add)
                    nc.vector.scalar_tensor_tensor(out=v_all[0:32, BC:BC + 1],
                                                   in0=wep[:, t:t + 1], scalar=1.0,
                                                   in1=wep[:, t:t + 1],
                                                   op0=Alu.is_lt, op1=Alu.mult)
                s2 = s2_pool.tile([32, K], f32, tag="s2st")
                nc.vector.tensor_scalar(s2[:], wep[:], 1.0, None, op0=Alu.is_ge)
                nc.sync.dma_start(out_ext[:, :, tau * K:(tau + 1) * K], s2[:])

    nc.compile()
    return nc


_program_cache = {}


def kernel(x, W1, W2):
    x = np.ascontiguousarray(np.asarray(x, dtype=np.float32))
    W1 = np.asarray(W1, dtype=np.float32)
    W2 = np.asarray(W2, dtype=np.float32)
    t_steps = x.shape[1]
    if t_steps not in _program_cache:
        _program_cache[t_steps] = build_program(t_steps)
    nc = _program_cache[t_steps]

    w1t = np.ascontiguousarray(W1.T)            # [d_in, d_out]
    w2t = np.ascontiguousarray(W2.T)            # [d_in, 2]
    in_maps = [
        {"x": np.ascontiguousarray(x[i * BC:(i + 1) * BC]), "w1t": w1t, "w2t": w2t}
        for i in range(NCORES)
    ]
    res = run_bass_kernel_spmd(nc, in_maps, list(range(NCORES)))
    # device layout is [O, BC, T]; full output is [B, T, O]
    outs = [np.transpose(np.asarray(res.results[i]["out"]), (1, 2, 0))
            for i in range(NCORES)]
    return np.ascontiguousarray(np.concatenate(outs, axis=0)).astype(np.float32)
